# revision 1
# baseline (speedup 1.0000x reference)
# DeepGEMM-style fp8 block-quantized linear for Trainium2, 8-core SPMD.
#
# reference semantics:
#   x_dq = dequant(quant_e4m3fn(x, per-token per-128-group amax/448 scales))
#   w_dq = w_fp8 * w_scale (per 128x128 block)
#   out  = (x_dq @ w_dq.T).astype(bf16)          # fp32 accumulation
#
# Strategy (per core; 2x4 [M x N] grid => M2=2048, N2=1024 per core):
#   - TRN fp8_e4m3 tops out at 240 (vs OCP e4m3fn's 448), so quantize
#     x * (112/amax) on TRN's grid: identical rounding for normals (pure
#     exponent shift); dequantize with s4 = amax/112.
#   - scales folded into fp16 matmul operands (fp16 keeps the e4m3fn
#     values and 448-max weight values exact, and halves bf16's rounding
#     noise); on-chip xbar DMA transposes into [K, *] layouts.
#   - quant/dequant/w-dequant are single big tensor_tensor ops using
#     step-0 (free-dim broadcast) access patterns for the per-128-group
#     scales; dequant runs on GpSimd to keep DVE under the PE roofline.
#   - psum [128, 256] quarter tiles accumulate over the 56 k-blocks;
#     n-quarter ordering lets matmuls start while W is still streaming in.

import numpy as np
import ml_dtypes
from contextlib import ExitStack

import concourse.bass as bass
import concourse.mybir as mybir
import concourse.tile as tile
from concourse import bacc
from concourse.bass_utils import run_bass_kernel_spmd

dt = mybir.dt

M, N, K = 4096, 4096, 7168
MSH, NSH = 2, 4                     # core grid: 2 along M, 4 along N
NCORES = MSH * NSH
BLK = 128


def bcast_inner(ap, n):
    """Append a step-0 inner dim of size n (free-dim broadcast read)."""
    return bass.AP(tensor=ap.tensor, offset=ap.offset, ap=[*ap.ap, [0, n]])


def emit_kernel(ctx, tc, o_d, x_d, w_d, ws_d, *, xq_engine="gpsimd", xdq_engine="vector", sc_engine="gpsimd", nq_width=256):
    nc = tc.nc
    f32, f16, f8 = dt.float32, dt.float16, dt.float8e4
    bf16 = dt.bfloat16
    M2, Kd = x_d.shape
    N2, _ = w_d.shape
    KB = Kd // BLK              # k-blocks
    NB = N2 // BLK              # n-blocks
    MT = M2 // BLK              # m-tiles
    NQ = N2 // nq_width         # psum tiles per m-tile
    KQ = 4                      # x pipeline chunks per m-tile
    KBQ = KB // KQ
    KHW = 2                     # w cast/transpose k-chunks per n-block
    assert KB % KQ == 0 and NB % 2 == 0

    wtp = ctx.enter_context(tc.tile_pool(name="wt", bufs=1))
    constp = ctx.enter_context(tc.tile_pool(name="consts", bufs=1))
    wqp = ctx.enter_context(tc.tile_pool(name="wq", bufs=2))
    xnp = ctx.enter_context(tc.tile_pool(name="xn", bufs=3))
    scp = ctx.enter_context(tc.tile_pool(name="sc", bufs=4))
    xqp = ctx.enter_context(tc.tile_pool(name="xq", bufs=2))
    xdqp = ctx.enter_context(tc.tile_pool(name="xdq", bufs=2))
    xtp = ctx.enter_context(tc.tile_pool(name="xt", bufs=10))
    obp = ctx.enter_context(tc.tile_pool(name="ob", bufs=2))
    psp = ctx.enter_context(tc.tile_pool(name="ps", bufs=2, space="PSUM"))

    # w_scale broadcast across partitions via step-0 partition DMA read
    wsb = constp.tile([128, NB * KB], f32)
    ws_flat = ws_d.rearrange("a b -> (a b)")
    ws_b = bass.AP(tensor=ws_flat.tensor, offset=ws_flat.offset,
                   ap=[[0, 128], *ws_flat.ap])
    nc.gpsimd.dma_start(wsb[:], ws_b)

    # stage 1: W -> wt_t[p, kb, n] = w[n, kb*128+p] * ws[n//128, kb]  (fp16)
    wt_t = wtp.tile([128, KB, N2], f16)
    KHL = Kd // KHW
    KBH = KB // KHW
    for nb in range(NB):
        for kh in range(KHW):
            wq = wqp.tile([128, KHL], f16, tag="wq")
            nc.gpsimd.dma_start(
                wq[:], w_d[nb * BLK:(nb + 1) * BLK, kh * KHL:(kh + 1) * KHL])
            nc.sync.dma_start(
                wt_t[:, kh * KBH:(kh + 1) * KBH, nb * BLK:(nb + 1) * BLK],
                wq[:], transpose=True)
            sl = wt_t[:, kh * KBH:(kh + 1) * KBH, nb * BLK:(nb + 1) * BLK]
            nc.vector.tensor_tensor(
                out=sl, in0=sl,
                in1=bcast_inner(
                    wsb[:, nb * KB + kh * KBH: nb * KB + (kh + 1) * KBH], BLK),
                op=mybir.AluOpType.mult)

    # stage 2: per m-tile quant + matmul
    xq_eng = getattr(nc, xq_engine)
    xdq_eng = getattr(nc, xdq_engine)
    sc_eng = getattr(nc, sc_engine)
    KL = Kd // KQ
    for mt in range(MT):
        xt_qs = []
        for q in range(KQ):
            xn = xnp.tile([128, KL], bf16, tag="xn")
            nc.sync.dma_start(xn[:], x_d[mt * BLK:(mt + 1) * BLK, q * KL:(q + 1) * KL])
            xng = xn[:].rearrange("p (kb c) -> p kb c", c=BLK)

            amax = scp.tile([128, KBQ], f32, tag="amax")
            nc.vector.reduce_max(
                amax[:], xng, axis=mybir.AxisListType.X, apply_absolute_value=True)
            # s4 ~= max(amax, 1e-12)/112 (== 4x reference scale up to 1 ulp)
            s4 = scp.tile([128, KBQ], f32, tag="s4")
            sc_eng.tensor_scalar(
                out=s4[:], in0=amax[:],
                scalar1=1e-12, scalar2=float(np.float32(1.0 / 112.0)),
                op0=mybir.AluOpType.max, op1=mybir.AluOpType.mult)
            inv4 = scp.tile([128, KBQ], f32, tag="inv4")
            nc.vector.reciprocal(inv4[:], s4[:])

            xq = xqp.tile([128, KL], f8, tag="xq")
            xqg = xq[:].rearrange("p (kb c) -> p kb c", c=BLK)
            xq_eng.tensor_tensor(
                out=xqg, in0=xng, in1=bcast_inner(inv4[:], BLK),
                op=mybir.AluOpType.mult)
            xdq = xdqp.tile([128, KL], f16, tag="xdq")
            xdqg = xdq[:].rearrange("p (kb c) -> p kb c", c=BLK)
            xdq_eng.tensor_tensor(
                out=xdqg, in0=xqg, in1=bcast_inner(s4[:], BLK),
                op=mybir.AluOpType.mult)

            xt_t = xtp.tile([128, KBQ, 128], f16, tag="xt")
            nc.sync.dma_start(xt_t[:], xdq[:], transpose=True)
            xt_qs.append(xt_t)

        ob = obp.tile([128, N2], bf16, tag="ob")
        for nq in range(NQ):
            pst = psp.tile([128, nq_width], f32, tag=f"ps{nq}")
            for kb in range(KB):
                nc.tensor.matmul(
                    pst[:],
                    xt_qs[kb // KBQ][:, kb % KBQ, :],
                    wt_t[:, kb, nq * nq_width:(nq + 1) * nq_width],
                    start=(kb == 0), stop=(kb == KB - 1))
            nc.scalar.copy(ob[:, nq * nq_width:(nq + 1) * nq_width], pst[:])
        nc.sync.dma_start(o_d[mt * BLK:(mt + 1) * BLK, :], ob[:])


def build_nc(m2, n2, k, **kw):
    nc = bacc.Bacc("TRN2", target_bir_lowering=False, debug=False, num_devices=NCORES)
    x_d = nc.dram_tensor("x", [m2, k], dt.bfloat16, kind="ExternalInput").ap()
    w_d = nc.dram_tensor("w", [n2, k], dt.float32, kind="ExternalInput").ap()
    ws_d = nc.dram_tensor("ws", [n2 // BLK, k // BLK], dt.float32, kind="ExternalInput").ap()
    o_d = nc.dram_tensor("o", [m2, n2], dt.bfloat16, kind="ExternalOutput").ap()
    with tile.TileContext(nc) as tc, ExitStack() as ctx:
        emit_kernel(ctx, tc, o_d, x_d, w_d, ws_d, **kw)
    nc.compile()
    return nc


_cache = {}


def _get_nc():
    if "nc" not in _cache:
        _cache["nc"] = build_nc(M // MSH, N // NSH, K)
    return _cache["nc"]


def kernel(input, weight_fp8, weight_scale, _trace=False, _trace_kwargs=None):
    input = np.asarray(input)
    if input.dtype != ml_dtypes.bfloat16:
        input = input.astype(ml_dtypes.bfloat16)
    weight_fp8 = np.asarray(weight_fp8, dtype=np.float32)
    weight_scale = np.asarray(weight_scale, dtype=np.float32)
    M2, N2 = M // MSH, N // NSH
    NSB = N2 // BLK

    in_maps = []
    for c in range(NCORES):
        mi, ni = divmod(c, NSH)
        in_maps.append({
            "x": np.ascontiguousarray(input[mi * M2:(mi + 1) * M2]),
            "w": np.ascontiguousarray(weight_fp8[ni * N2:(ni + 1) * N2]),
            "ws": np.ascontiguousarray(weight_scale[ni * NSB:(ni + 1) * NSB]),
        })

    nc = _get_nc()
    kw = {}
    if _trace:
        kw = dict(trace=True, **(_trace_kwargs or {}))
    res = run_bass_kernel_spmd(nc, in_maps, core_ids=list(range(NCORES)), **kw)

    out = np.empty((M, N), dtype=ml_dtypes.bfloat16)
    for c in range(NCORES):
        mi, ni = divmod(c, NSH)
        out[mi * M2:(mi + 1) * M2, ni * N2:(ni + 1) * N2] = res.results[c]["o"]
    if _trace:
        return out, res
    return out

